# revision 25
# baseline (speedup 1.0000x reference)
"""Trainium2 Bass kernel for nn_DWMF_51874615001833 (sparse patch attention).

Computation (matches the reference nn.Module):
  - x[0:4]  : 4 local images [256, 80, 80], split into 64 patches of [256, 20, 20]
  - x[4]    : global image, bilinear-upsampled 2x and split into an 8x8 patch grid
  - per patch: head-averaged, query-averaged attention weights between the
    (local patch + sine pos-embed) queries and (global patch + pos-embed) keys
    produce a [20, 20] importance map; the local patch is scaled by
    0.6 * sigmoid(sal_w * imp + sal_b), patches are re-assembled, plus
    0.5 * upsampled-global.

Device/host split: the device computes ONLY the attention reduction
  colsum[s] = sum_h sum_l exp(score_h[l, s]) / rowsum_h[l]
per patch (the compute-bound core: 64 x 8 heads x 400x400 softmax weights).
Everything that is cheap elementwise host work rides on the host: sine pos
embedding (folded into the inputs), the 2x bilinear upsample of the global
image (the key patches are shipped pre-upsampled), the final
sigmoid / multiply / merge. The device inputs are fp8e4 (validated: final
rel err ~3e-6 vs fp32 reference), which also halves/quarters DMA traffic.

Sharding: core d handles patches n = 8d..8d+7 (data parallel), which attend
to row d of the 8x8 global key grid. Projection weights are replicated.

Per-core device pipeline (8 patches):
  - qp/kp = W^T @ (x + pos) + b : one fp8 DoubleRow matmul per (proj, mt)
    half (contracts all 256 input dims in one instruction at 0.5 cyc/row),
    bias added during the PSUM->SBUF move (DVE tensor_scalar with
    per-partition scalar), output fp32r so the score matmuls self-load.
  - scores per (l-tile, head): PE fp32r matmuls, K=32, quadrant-packed via
    tile_position. The 16-row l-tail of all 4 head pairs is packed into ONE
    [128, 2, 400] PSUM tile (pairs at partitions 32g) so its exp costs one
    instruction instead of four (ACT is the bottleneck engine).
  - exp on ACT (two heads per instruction), bf16 out.
  - rowsums on DVE (tensor_scalar accum_out, 4x perf mode: all-SBUF bf16).
  - colsum via PE matvec accumulation with fp32r rinv weights (self-loading
    matmuls: no Ldweights sequencer cost), bf16 moving data, accumulated
    across all (head, l-tile) into one [1, 400] PSUM row per patch; 4
    patches share one PSUM bank at partitions {0, 32, 64, 96}.
  - two [4, 400] DMAs ship the raw colsums out; host applies the sigmoid.
"""

import math

import numpy as np

import concourse.bass as bass
import concourse.tile as tile
from concourse import bacc, mybir
from concourse.bass_utils import run_bass_kernel_spmd

FP32 = mybir.dt.float32
R32 = mybir.dt.float32r
BF16 = mybir.dt.bfloat16
FP8 = mybir.dt.float8e4
DR = mybir.MatmulPerfMode.DoubleRow


def _r(ap):
    return ap.bitcast(R32)


ALU = mybir.AluOpType
ACTF = mybir.ActivationFunctionType

D = 256
NHEADS = 8
PH = PW = 20
L = S = 400
LT = [(0, 128), (128, 128), (256, 128)]  # full l-tiles; 16-row tail packed


# ----------------------------------------------------------------------------
# Host-side preparation
# ----------------------------------------------------------------------------

def _pos_embed_sine(h, w, F):
    scale = 2.0 * math.pi
    eps = 1e-6
    y = (np.arange(1, h + 1, dtype=np.float64) - 0.5) / (h + eps) * scale
    x = (np.arange(1, w + 1, dtype=np.float64) - 0.5) / (w + eps) * scale
    i = np.arange(F, dtype=np.float64)
    dim_t = 10000.0 ** (2.0 * np.floor(i / 2.0) / F)
    px = x[:, None] / dim_t
    py = y[:, None] / dim_t

    def interleave(p):
        return np.stack(
            [np.sin(p[:, 0::2]), np.cos(p[:, 1::2])], axis=-1
        ).reshape(p.shape[0], -1)

    px = interleave(px)
    py = interleave(py)
    pos_y = np.broadcast_to(py[:, None, :], (h, w, F))
    pos_x = np.broadcast_to(px[None, :, :], (h, w, F))
    return np.concatenate([pos_y, pos_x], axis=-1).transpose(2, 0, 1)  # [2F,h,w]


def _up2(a):
    """Separable 2x bilinear upsample (align_corners=False): [C,H,W]->[C,2H,2W]."""
    C, H, W = a.shape
    pad = np.pad(a, ((0, 0), (1, 1), (1, 1)), mode="edge")
    h0 = 0.75 * pad[:, 1:1 + H, :] + 0.25 * pad[:, 0:H, :]
    h1 = 0.75 * pad[:, 1:1 + H, :] + 0.25 * pad[:, 2:2 + H, :]
    v = np.empty((C, 2 * H, W + 2), a.dtype)
    v[:, 0::2] = h0
    v[:, 1::2] = h1
    out = np.empty((C, 2 * H, 2 * W), a.dtype)
    out[:, :, 0::2] = 0.75 * v[:, :, 1:1 + W] + 0.25 * v[:, :, 0:W]
    out[:, :, 1::2] = 0.75 * v[:, :, 1:1 + W] + 0.25 * v[:, :, 2:2 + W]
    return out


def _fp8(a):
    return np.ascontiguousarray(a.astype(mybir.dt.np(FP8)))


def prepare_inputs(x, in_proj_w, in_proj_b, sal_w, sal_b):
    """Returns (in_maps list of 8 dicts, post ctx dict for finish_output)."""
    x = np.asarray(x, np.float32)
    in_proj_w = np.asarray(in_proj_w, np.float32)
    in_proj_b = np.asarray(in_proj_b, np.float32)
    inv = 1.0 / math.sqrt(D // NHEADS)

    pos = _pos_embed_sine(PH, PW, D // 2).reshape(D, L).astype(np.float32)
    Wq = in_proj_w[:D].astype(np.float64)
    Wk = in_proj_w[D:2 * D].astype(np.float64)
    bq = in_proj_b[:D].astype(np.float64)
    bk = in_proj_b[D:2 * D].astype(np.float64)

    # weights e-split [128(e-lo), 2(e-hi), 256(m)] for DoubleRow contraction
    wq_dev = _fp8((Wq.T * inv).reshape(2, 128, D).transpose(1, 0, 2))
    wk_dev = _fp8(Wk.T.reshape(2, 128, D).transpose(1, 0, 2))
    # per-feature bias columns [128(m-lo), 2(mt)]
    bqc = np.ascontiguousarray((bq * inv).reshape(2, 128).T.astype(np.float32))
    bkc = np.ascontiguousarray(bk.reshape(2, 128).T.astype(np.float32))

    loc = x[:4]
    upg = _up2(x[4])  # [256, 160, 160]
    locp = (loc.reshape(4, D, 4, PH, 4, PW).transpose(0, 2, 4, 1, 3, 5)
            .reshape(64, D, L))
    glbp = (upg.reshape(D, 8, PH, 8, PW).transpose(1, 3, 0, 2, 4)
            .reshape(64, D, L))

    # e-split + pos fold, per patch: [64, 128, 2, 400]
    qall = (locp + pos).reshape(64, 2, 128, L).transpose(0, 2, 1, 3)
    kall = (glbp + pos).reshape(64, 2, 128, L).transpose(0, 2, 1, 3)
    qall = _fp8(qall)
    kall = _fp8(kall)

    in_maps = []
    for d in range(8):
        in_maps.append({
            "xq": np.ascontiguousarray(qall[8 * d:8 * d + 8]),  # [8,128,2,400]
            "ukey": np.ascontiguousarray(
                kall[8 * d:8 * d + 8].transpose(1, 2, 0, 3)),   # [128,2,8,400]
            "wq": wq_dev, "wk": wk_dev, "bqc": bqc, "bkc": bkc,
            "qz": np.zeros((128, 2, 64), np.float32),
        })
    post = {
        "locp": locp.reshape(64, D, PH, PW),
        "upg": upg,
        "sal_w": float(np.asarray(sal_w).reshape(-1)[0]),
        "sal_b": float(np.asarray(sal_b).reshape(-1)[0]),
    }
    return in_maps, post


def finish_output(per_core_outs, post):
    """per_core_outs: list of 8 arrays [4, 2, 400] (raw colsums, [q, t] =
    patch 4t+q) -> full output."""
    cs = np.empty((64, L), np.float64)
    for d in range(8):
        o = np.asarray(per_core_outs[d], np.float64)  # [4, 2, 400]
        cs[8 * d:8 * d + 8] = o.transpose(1, 0, 2).reshape(8, L)
    z = post["sal_w"] * cs / (NHEADS * L) + post["sal_b"]
    imp = (0.6 / (1.0 + np.exp(-z))).astype(np.float32)  # [64, 400]
    weighted = post["locp"] * imp.reshape(64, 1, PH, PW)
    loc_m = (weighted.reshape(4, 4, 4, D, PH, PW)
             .transpose(0, 3, 1, 4, 2, 5).reshape(4, D, 80, 80))
    merged = (loc_m.reshape(2, 2, 1, D, 80, 80)
              .transpose(2, 3, 0, 4, 1, 5).reshape(1, D, 160, 160))
    return merged + post["upg"][None] * 0.5


# ----------------------------------------------------------------------------
# Device kernel
# ----------------------------------------------------------------------------

def _body(nc, tc, pools, aps, lag=2):
    const, xq_p, qk_p, e_p, r_p, ps, pt_p, pcb = pools
    xqd, ukeyd, wqd, wkd, bqcd, bkcd, qzd, outp = aps

    # First-patch inputs first; weights on the ACT engine's DMA queue.
    xq0 = xq_p.tile([128, 2, 400], FP8, tag="xq")
    nc.sync.dma_start(xq0[:], xqd[0])
    ukA = const.tile([128, 2, 4, 400], FP8, tag="ukA")
    ukB = const.tile([128, 2, 4, 400], FP8, tag="ukB")
    nc.sync.dma_start(ukA[:, :, 0:1], ukeyd[:, :, 0:1])
    wq_s = const.tile([128, 2, 256], FP8, tag="wq")
    wk_s = const.tile([128, 2, 256], FP8, tag="wk")
    bqc_s = const.tile([128, 2], FP32, tag="bqc")
    bkc_s = const.tile([128, 2], FP32, tag="bkc")
    nc.scalar.dma_start(wq_s[:], wqd)
    nc.scalar.dma_start(wk_s[:], wkd)
    nc.scalar.dma_start(bqc_s[:], bqcd)
    nc.scalar.dma_start(bkc_s[:], bkcd)
    # zero-padded tail lhsT scaffold (fp32r memset is invalid ISA; DMA zeros)
    qt = const.tile([128, 2, 64], R32, tag="qt")
    nc.scalar.dma_start(qt[:], qzd)
    nc.sync.dma_start(ukA[:, :, 1:4], ukeyd[:, :, 1:4])
    nc.sync.dma_start(ukB[:], ukeyd[:, :, 4:8])

    scr = const.tile([128, 400], BF16, tag="scr")  # write-only rowsum main out
    stage = const.tile([128, 2, 400], FP32, tag="stage")  # colsum staging

    pcbt = [pcb.tile([128, 512], FP32, tag="pcb", bufs=1, name=f"pcb{t}")
            for t in range(2)]

    P = {}  # per-patch state

    def emit_xq(j):
        if j == 0:
            xq = xq0
        else:
            xq = xq_p.tile([128, 2, 400], FP8, tag="xq")
            nc.sync.dma_start(xq[:], xqd[j])
        P[j] = dict(xq=xq, esbs={}, rinvs={})

    def emit_proj_part(j, part):
        # One (projection, mt) quarter; spread across the previous patch's
        # blocks. A single fp8 DoubleRow matmul contracts all 256 input dims.
        is_q, mt = ((True, 0), (False, 0), (True, 1), (False, 1))[part]
        w_s, b_s = (wq_s, bqc_s) if is_q else (wk_s, bkc_s)
        if is_q:
            rhs = P[j]["xq"][:]
        else:
            uk = ukA if j < 4 else ukB
            rhs = uk[:, :, j % 4]
        pt_t = pt_p.tile([128, 512], FP32, tag="pt", bufs=1)
        nc.tensor.matmul(pt_t[:, 0:400], lhsT=w_s[:, :, 128 * mt:128 * mt + 128],
                         rhs=rhs, start=True, stop=True, perf_mode=DR)
        key = "qp" if is_q else "kp"
        if mt == 0:
            P[j][key] = qk_p.tile([128, 2, 400], R32, tag=key, name=key)
        # bias add during the PSUM->SBUF move (per-partition scalar)
        nc.vector.tensor_scalar(
            out=P[j][key][:, mt, :], in0=pt_t[:, 0:400],
            scalar1=b_s[:, mt:mt + 1], scalar2=None, op0=ALU.add)

    def emit_block(j, lt):
        qp, kp = P[j]["qp"], P[j]["kp"]
        if lt == 2:
            # Packed 16-row l-tail: ONE matmul per e-half computes all 4 of
            # that half's heads' tail scores as an M=64 output at partition
            # base 0 (tail row p = 16*hq + (l-384) for head 4*half+hq) — the
            # only PE tile family walrus encodes reliably. The lhsT is the
            # zero-padded qt const (head hq's 32 k-rows hold its qp tail,
            # other rows zero), so the full-K contraction of kp is exact.
            # Emitted ahead of block lt2's scores so the tail exp (block lt3)
            # never waits on them.
            for h in range(NHEADS):
                hq, half = h % 4, h // 4
                nc.vector.tensor_scalar_add(
                    qt[32 * hq:32 * hq + 32, half, 16 * hq:16 * hq + 16],
                    qp[32 * hq:32 * hq + 32, half, 384:400], 0.0)
            stt = ps.tile([128, 2, 512], FP32, tag="pstail", bufs=1, name="stt")
            for half in range(2):
                nc.tensor.matmul(
                    stt[0:64, half, 0:400], lhsT=qt[:, half, :],
                    rhs=kp[:, half, :], start=True, stop=True)
            P[j]["stt"] = stt
        if lt < 3:
            l0, lsz = LT[lt]
            rs = r_p.tile([128, 8], FP32, tag="rs")
            for g in range(4):
                st = ps.tile([128, 2, 512], FP32, tag="ps")
                for i in range(2):
                    h = 2 * g + i
                    ab = 32 * (h % 4)
                    nc.tensor.matmul(
                        st[:lsz, i, 0:400],
                        lhsT=qp[ab:ab + 32, h // 4, l0:l0 + lsz],
                        rhs=kp[ab:ab + 32, h // 4, :],
                        start=True, stop=True, tile_position=(ab, 0))
                et = e_p.tile([128, 2, 400], BF16, tag="e")
                nc.scalar.activation(out=et[:lsz, :, :],
                                     in_=st[:lsz, :, 0:400], func=ACTF.Exp)
                for i in range(2):
                    h = 2 * g + i
                    P[j]["esbs"][(lt, h)] = et[:, i, :]
                    nc.vector.tensor_scalar(
                        out=scr[:lsz, :], in0=et[:lsz, i, :], scalar1=0.0,
                        scalar2=0.0, op0=ALU.add, op1=ALU.add,
                        accum_out=rs[:lsz, h:h + 1])
            ri = r_p.tile([128, 8], BF16, tag="ri")
            nc.vector.reciprocal(ri[:lsz, :], rs[:lsz, :])
            P[j]["rinvs"][lt] = ri
        else:
            # Tail block: one exp covers both halves' packed tails. PSUM rows
            # 64..128 of the pstail banks are never written (they stay 0 from
            # reset, exp gives 1.0) and their rinv rows are memset to zero so
            # the colsum ignores them.
            stt = P[j]["stt"]
            et = e_p.tile([128, 2, 400], BF16, tag="e")
            nc.scalar.activation(out=et[:], in_=stt[:, :, 0:400], func=ACTF.Exp)
            P[j]["etail"] = et
            rit = r_p.tile([128, 2], BF16, tag="rit")
            nc.vector.memset(rit[:], 0.0)
            rst = r_p.tile([128, 2], FP32, tag="rst")
            for half in range(2):
                nc.vector.tensor_scalar(
                    out=scr[0:64, :], in0=et[0:64, half, :], scalar1=0.0,
                    scalar2=0.0, op0=ALU.add, op1=ALU.add,
                    accum_out=rst[0:64, half:half + 1])
            nc.vector.reciprocal(rit[0:64, :], rst[0:64, :])
            P[j]["ritail"] = rit

    def emit_colsums(j, lt):
        row = pcbt[j // 4][32 * (j % 4):32 * (j % 4) + 1, 0:400]
        tp = (0, 32 * (j % 4))
        if lt < 3:
            l0, lsz = LT[lt]
            for h in range(NHEADS):
                nc.tensor.matmul(
                    row, lhsT=P[j]["rinvs"][lt][:lsz, h:h + 1],
                    rhs=P[j]["esbs"][(lt, h)][:lsz, :],
                    start=(lt == 0 and h == 0), stop=False, tile_position=tp)
        else:
            for half in range(2):
                nc.tensor.matmul(
                    row, lhsT=P[j]["ritail"][:, half:half + 1],
                    rhs=P[j]["etail"][:, half, :],
                    start=False, stop=(half == 1), tile_position=tp)
            del P[j]

    blocks = [(j, lt) for j in range(8) for lt in range(4)]
    n = len(blocks)
    emit_xq(0)
    for part in range(4):
        emit_proj_part(0, part)
    for idx in range(n + lag):
        if idx < n:
            j, lt = blocks[idx]
            emit_block(j, lt)
            if lt == 0 and j + 1 < 8:
                emit_xq(j + 1)
            if j + 1 < 8:
                emit_proj_part(j + 1, lt)
        if lag <= idx:
            jc, ltc = blocks[idx - lag]
            emit_colsums(jc, ltc)
            if ltc == 3:
                t, q = jc // 4, jc % 4
                nc.vector.tensor_scalar_add(
                    stage[32 * q:32 * q + 1, t, :],
                    pcbt[t][32 * q:32 * q + 1, 0:400], 0.0)
                if jc == 7:
                    src = stage.rearrange("(q s) t f -> q s t f", s=32)[:, 0]
                    nc.sync.dma_start(outp, src)


def build(reps=1, lag=2):
    nc = bacc.Bacc("TRN2", target_bir_lowering=False, debug=False, num_devices=8)
    xqd = nc.dram_tensor("xq", (8, 128, 2, 400), FP8, kind="ExternalInput").ap()
    ukeyd = nc.dram_tensor("ukey", (128, 2, 8, 400), FP8, kind="ExternalInput").ap()
    wqd = nc.dram_tensor("wq", (128, 2, 256), FP8, kind="ExternalInput").ap()
    wkd = nc.dram_tensor("wk", (128, 2, 256), FP8, kind="ExternalInput").ap()
    bqcd = nc.dram_tensor("bqc", (128, 2), FP32, kind="ExternalInput").ap()
    bkcd = nc.dram_tensor("bkc", (128, 2), FP32, kind="ExternalInput").ap()
    qzd = nc.dram_tensor("qz", (128, 2, 64), R32, kind="ExternalInput").ap()
    outp = nc.dram_tensor("out", (4, 2, 400), FP32, kind="ExternalOutput").ap()
    aps = (xqd, ukeyd, wqd, wkd, bqcd, bkcd, qzd, outp)

    with tile.TileContext(nc) as tc:
        with (
            tc.tile_pool(name="const", bufs=1) as const,
            tc.tile_pool(name="xq", bufs=3) as xq_p,
            tc.tile_pool(name="qk", bufs=4) as qk_p,
            tc.tile_pool(name="e", bufs=17) as e_p,
            tc.tile_pool(name="r", bufs=8) as r_p,
            tc.tile_pool(name="ps", bufs=2, space="PSUM") as ps,
            tc.tile_pool(name="pt", bufs=1, space="PSUM") as pt_p,
            tc.tile_pool(name="pcb", bufs=1, space="PSUM") as pcb,
        ):
            pools = (const, xq_p, qk_p, e_p, r_p, ps, pt_p, pcb)
            with nc.allow_low_precision(reason="fp8/bf16/fp32r attention"):
                if reps == 1:
                    _body(nc, tc, pools, aps, lag)
                else:
                    with tc.For_i(0, reps, 1):
                        _body(nc, tc, pools, aps, lag)
    nc.compile()
    return nc


# ----------------------------------------------------------------------------
# Entry point
# ----------------------------------------------------------------------------

def kernel(**inputs) -> np.ndarray:
    in_maps, post = prepare_inputs(
        inputs["x"], inputs["in_proj_w"], inputs["in_proj_b"],
        inputs["sal_w"], inputs["sal_b"])
    nc = build(reps=1)
    res = run_bass_kernel_spmd(nc, in_maps, core_ids=list(range(8)))
    return finish_output([r["out"] for r in res.results], post).astype(np.float32)


if __name__ == "__main__":
    rng = np.random.default_rng(0)
    ins = {
        "x": rng.standard_normal((5, 256, 80, 80), dtype=np.float32),
        "in_proj_w": (rng.standard_normal((768, 256)) * 0.05).astype(np.float32),
        "in_proj_b": (rng.standard_normal(768) * 0.05).astype(np.float32),
        "sal_w": rng.standard_normal(1).astype(np.float32),
        "sal_b": rng.standard_normal(1).astype(np.float32),
    }
    out = kernel(**ins)
    print("kernel out:", out.shape, out.dtype, float(np.abs(out).mean()))


# revision 37
# speedup vs baseline: 1.1741x; 1.1741x over previous
"""Trainium2 Bass kernel for nn_DWMF_51874615001833 (sparse patch attention).

Computation (matches the reference nn.Module):
  - x[0:4]  : 4 local images [256, 80, 80], split into 64 patches of [256, 20, 20]
  - x[4]    : global image, bilinear-upsampled 2x and split into an 8x8 patch grid
  - per patch: head-averaged, query-averaged attention weights between the
    (local patch + sine pos-embed) queries and (global patch + pos-embed) keys
    produce a [20, 20] importance map; the local patch is scaled by
    0.6 * sigmoid(sal_w * imp + sal_b), patches are re-assembled, plus
    0.5 * upsampled-global.

Device/host split: the device computes ONLY the attention reduction
  colsum[s] = sum_h sum_l exp(score_h[l, s]) / rowsum_h[l]
per patch (the compute-bound core: 64 x 8 heads x 400x400 softmax weights).
Everything that is cheap elementwise host work rides on the host: sine pos
embedding (folded into the inputs), the 2x bilinear upsample of the global
image (the key patches are shipped pre-upsampled), the final
sigmoid / multiply / merge. The device inputs are fp8e4 (validated: final
rel err ~3e-6 vs fp32 reference), which also halves/quarters DMA traffic.

Sharding: core d handles patches n = 8d..8d+7 (data parallel), which attend
to row d of the 8x8 global key grid. Projection weights are replicated.

Per-core device pipeline (8 patches):
  - qp/kp = W^T @ (x + pos) + b : one fp8 DoubleRow matmul per (proj, mt)
    half (contracts all 256 input dims in one instruction at 0.5 cyc/row),
    bias added during the PSUM->SBUF move (DVE tensor_scalar with
    per-partition scalar), output fp32r so the score matmuls self-load.
  - scores per (l-tile, head): PE fp32r matmuls, K=32, quadrant-packed via
    tile_position. The 16-row l-tail of all 4 head pairs is packed into ONE
    [128, 2, 400] PSUM tile (pairs at partitions 32g) so its exp costs one
    instruction instead of four (ACT is the bottleneck engine).
  - exp on ACT (two heads per instruction), bf16 out.
  - rowsums on DVE (tensor_scalar accum_out, 4x perf mode: all-SBUF bf16).
  - colsum via PE matvec accumulation with fp32r rinv weights (self-loading
    matmuls: no Ldweights sequencer cost), bf16 moving data, accumulated
    across all (head, l-tile) into one [1, 400] PSUM row per patch; 4
    patches share one PSUM bank at partitions {0, 32, 64, 96}.
  - two [4, 400] DMAs ship the raw colsums out; host applies the sigmoid.
"""

import math

import numpy as np

import concourse.bass as bass
import concourse.tile as tile
from concourse import bacc, mybir
from concourse.bass_utils import run_bass_kernel_spmd

FP32 = mybir.dt.float32
R32 = mybir.dt.float32r
BF16 = mybir.dt.bfloat16
FP8 = mybir.dt.float8e4
DR = mybir.MatmulPerfMode.DoubleRow


def _r(ap):
    return ap.bitcast(R32)


ALU = mybir.AluOpType
ACTF = mybir.ActivationFunctionType

D = 256
NHEADS = 8
PH = PW = 20
L = S = 400
LT = [(0, 128), (128, 128), (256, 128)]  # full l-tiles; 16-row tail packed


# ----------------------------------------------------------------------------
# Host-side preparation
# ----------------------------------------------------------------------------

def _pos_embed_sine(h, w, F):
    scale = 2.0 * math.pi
    eps = 1e-6
    y = (np.arange(1, h + 1, dtype=np.float64) - 0.5) / (h + eps) * scale
    x = (np.arange(1, w + 1, dtype=np.float64) - 0.5) / (w + eps) * scale
    i = np.arange(F, dtype=np.float64)
    dim_t = 10000.0 ** (2.0 * np.floor(i / 2.0) / F)
    px = x[:, None] / dim_t
    py = y[:, None] / dim_t

    def interleave(p):
        return np.stack(
            [np.sin(p[:, 0::2]), np.cos(p[:, 1::2])], axis=-1
        ).reshape(p.shape[0], -1)

    px = interleave(px)
    py = interleave(py)
    pos_y = np.broadcast_to(py[:, None, :], (h, w, F))
    pos_x = np.broadcast_to(px[None, :, :], (h, w, F))
    return np.concatenate([pos_y, pos_x], axis=-1).transpose(2, 0, 1)  # [2F,h,w]


def _up2(a):
    """Separable 2x bilinear upsample (align_corners=False): [C,H,W]->[C,2H,2W]."""
    C, H, W = a.shape
    pad = np.pad(a, ((0, 0), (1, 1), (1, 1)), mode="edge")
    h0 = 0.75 * pad[:, 1:1 + H, :] + 0.25 * pad[:, 0:H, :]
    h1 = 0.75 * pad[:, 1:1 + H, :] + 0.25 * pad[:, 2:2 + H, :]
    v = np.empty((C, 2 * H, W + 2), a.dtype)
    v[:, 0::2] = h0
    v[:, 1::2] = h1
    out = np.empty((C, 2 * H, 2 * W), a.dtype)
    out[:, :, 0::2] = 0.75 * v[:, :, 1:1 + W] + 0.25 * v[:, :, 0:W]
    out[:, :, 1::2] = 0.75 * v[:, :, 1:1 + W] + 0.25 * v[:, :, 2:2 + W]
    return out


def _fp8(a):
    return np.ascontiguousarray(a.astype(mybir.dt.np(FP8)))


def prepare_inputs(x, in_proj_w, in_proj_b, sal_w, sal_b):
    """Returns (in_maps list of 8 dicts, post ctx dict for finish_output)."""
    x = np.asarray(x, np.float32)
    in_proj_w = np.asarray(in_proj_w, np.float32)
    in_proj_b = np.asarray(in_proj_b, np.float32)
    inv = 1.0 / math.sqrt(D // NHEADS)

    pos = _pos_embed_sine(PH, PW, D // 2).reshape(D, L).astype(np.float32)
    Wq = in_proj_w[:D].astype(np.float64)
    Wk = in_proj_w[D:2 * D].astype(np.float64)
    bq = in_proj_b[:D].astype(np.float64)
    bk = in_proj_b[D:2 * D].astype(np.float64)

    # weights e-split [128(e-lo), 2(e-hi), 256(m)] for DoubleRow contraction
    wq_dev = _fp8((Wq.T * inv).reshape(2, 128, D).transpose(1, 0, 2))
    wk_dev = _fp8(Wk.T.reshape(2, 128, D).transpose(1, 0, 2))
    # per-feature bias columns [128(m-lo), 2(mt)]
    bqc = np.ascontiguousarray((bq * inv).reshape(2, 128).T.astype(np.float32))
    bkc = np.ascontiguousarray(bk.reshape(2, 128).T.astype(np.float32))

    loc = x[:4]
    upg = _up2(x[4])  # [256, 160, 160]
    locp = (loc.reshape(4, D, 4, PH, 4, PW).transpose(0, 2, 4, 1, 3, 5)
            .reshape(64, D, L))
    glbp = (upg.reshape(D, 8, PH, 8, PW).transpose(1, 3, 0, 2, 4)
            .reshape(64, D, L))

    # e-split + pos fold, per patch: [64, 128, 2, 400]
    qall = (locp + pos).reshape(64, 2, 128, L).transpose(0, 2, 1, 3)
    kall = (glbp + pos).reshape(64, 2, 128, L).transpose(0, 2, 1, 3)
    qall = _fp8(qall)
    kall = _fp8(kall)

    wqk = np.concatenate([wq_dev, wk_dev], axis=2)          # [128, 2, 512]
    misc = np.concatenate(
        [bqc, bkc, np.zeros((128, 256), np.float32)], axis=1)  # [128, 260]
    in_maps = []
    for d in range(8):
        in_maps.append({
            "xq": np.ascontiguousarray(qall[8 * d:8 * d + 8]),  # [8,128,2,400]
            "ukey": np.ascontiguousarray(
                kall[8 * d:8 * d + 8].transpose(1, 2, 0, 3)),   # [128,2,8,400]
            "wqk": wqk, "misc": misc,
        })
    post = {
        "locp": locp.reshape(64, D, PH, PW),
        "upg": upg,
        "sal_w": float(np.asarray(sal_w).reshape(-1)[0]),
        "sal_b": float(np.asarray(sal_b).reshape(-1)[0]),
    }
    return in_maps, post


def finish_output(per_core_outs, post):
    """per_core_outs: list of 8 arrays [4, 2, 400] (raw colsums, [q, t] =
    patch 4t+q) -> full output."""
    cs = np.empty((64, L), np.float64)
    for d in range(8):
        o = np.asarray(per_core_outs[d], np.float64)  # [4, 2, 400]
        cs[8 * d:8 * d + 8] = o.transpose(1, 0, 2).reshape(8, L)
    z = post["sal_w"] * cs / (NHEADS * L) + post["sal_b"]
    imp = (0.6 / (1.0 + np.exp(-z))).astype(np.float32)  # [64, 400]
    weighted = post["locp"] * imp.reshape(64, 1, PH, PW)
    loc_m = (weighted.reshape(4, 4, 4, D, PH, PW)
             .transpose(0, 3, 1, 4, 2, 5).reshape(4, D, 80, 80))
    merged = (loc_m.reshape(2, 2, 1, D, 80, 80)
              .transpose(2, 3, 0, 4, 1, 5).reshape(1, D, 160, 160))
    return merged + post["upg"][None] * 0.5


# ----------------------------------------------------------------------------
# Device kernel
# ----------------------------------------------------------------------------

def _body(nc, tc, pools, aps, lag=2):
    const, xq_p, qk_p, e_p, r_p, ps, pt_p, pcb = pools
    xqd, ukeyd, wqkd, miscd, outp = aps

    # Startup-critical DMAs first, alternating queues (descriptor generation
    # is serial across queues): xq0 | wqk | ukey[0] gate the first scores.
    # Weights/biases/zeros are merged into single transfers to cut descriptor
    # generations.
    xq0 = xq_p.tile([128, 2, 400], FP8, tag="xq")
    nc.sync.dma_start(xq0[:], xqd[0])
    wqk = const.tile([128, 2, 512], FP8, tag="wqk")
    nc.scalar.dma_start(wqk[:], wqkd)
    wq_s = wqk[:, :, 0:256]
    wk_s = wqk[:, :, 256:512]
    ukA = const.tile([128, 2, 4, 400], FP8, tag="ukA")
    ukB = const.tile([128, 2, 4, 400], FP8, tag="ukB")
    nc.sync.dma_start(ukA[:, :, 0:1], ukeyd[:, :, 0:1])
    # misc: [0:2] q bias cols, [2:4] k bias cols, [4:132] the zero-padded
    # tail-lhsT scaffold (fp32r memset is invalid ISA, so zeros ride the DMA)
    misc = const.tile([128, 260], R32, tag="misc")
    nc.scalar.dma_start(misc[:], miscd)
    bqc_s = misc[:, 0:2].bitcast(FP32)
    bkc_s = misc[:, 2:4].bitcast(FP32)
    qt = misc[:, 4:260].rearrange("p (a b) -> p a b", a=2)

    scr = const.tile([128, 400], BF16, tag="scr")  # write-only rowsum main out
    stage = const.tile([128, 2, 400], FP32, tag="stage")  # colsum staging

    pcbt = [pcb.tile([128, 512], FP32, tag="pcb", bufs=1, name=f"pcb{t}")
            for t in range(2)]

    P = {}  # per-patch state

    def emit_xq(j):
        if j == 0:
            xq = xq0
        else:
            xq = xq_p.tile([128, 2, 400], FP8, tag="xq")
            nc.sync.dma_start(xq[:], xqd[j])
        P[j] = dict(xq=xq, esbs={}, rinvs={})

    def emit_proj_part(j, part, use_ps=False):
        # One (projection, mt) quarter; spread across the previous patch's
        # blocks. A single fp8 DoubleRow matmul contracts all 256 input dims.
        # Patch 0's parts draw PSUM from the still-empty score ring so they
        # pipeline in parallel banks instead of serializing on the single
        # pt bank (startup latency).
        is_q, mt = ((True, 0), (False, 0), (True, 1), (False, 1))[part]
        w_s, b_s = (wq_s, bqc_s) if is_q else (wk_s, bkc_s)
        if is_q:
            rhs = P[j]["xq"][:]
        else:
            uk = ukA if j < 4 else ukB
            rhs = uk[:, :, j % 4]
        if use_ps:
            pt_t = ps.tile([128, 2, 512], FP32, tag="ps", name="pt_t")[:, 0, :]
        else:
            pt_t = pt_p.tile([128, 512], FP32, tag="pt", bufs=1)
        nc.tensor.matmul(pt_t[:, 0:400], lhsT=w_s[:, :, 128 * mt:128 * mt + 128],
                         rhs=rhs, start=True, stop=True, perf_mode=DR)
        key = "qp" if is_q else "kp"
        if mt == 0:
            P[j][key] = qk_p.tile([128, 2, 400], R32, tag=key, name=key)
        # bias add during the PSUM->SBUF move (per-partition scalar)
        nc.vector.tensor_scalar(
            out=P[j][key][:, mt, :], in0=pt_t[:, 0:400],
            scalar1=b_s[:, mt:mt + 1], scalar2=None, op0=ALU.add)

    def emit_block(j, lt):
        qp, kp = P[j]["qp"], P[j]["kp"]
        if lt == 2:
            # Packed 16-row l-tail: ONE matmul per e-half computes all 4 of
            # that half's heads' tail scores as an M=64 output at partition
            # base 0 (tail row p = 16*hq + (l-384) for head 4*half+hq) — the
            # only PE tile family walrus encodes reliably. The lhsT is the
            # zero-padded qt const (head hq's 32 k-rows hold its qp tail,
            # other rows zero), so the full-K contraction of kp is exact.
            # Emitted ahead of block lt2's scores so the tail exp (block lt3)
            # never waits on them.
            for h in range(NHEADS):
                hq, half = h % 4, h // 4
                nc.vector.tensor_scalar_add(
                    qt[32 * hq:32 * hq + 32, half, 16 * hq:16 * hq + 16],
                    qp[32 * hq:32 * hq + 32, half, 384:400], 0.0)
            stt = ps.tile([128, 2, 512], FP32, tag="pstail", bufs=1, name="stt")
            for half in range(2):
                # M=128: columns 64..128 of qt are zero, so PSUM rows 64..128
                # are WRITTEN (to 0) rather than left stale -- exp(0)=1 is
                # finite and those rows' rinv is zeroed. Same 400-column cost.
                nc.tensor.matmul(
                    stt[:, half, 0:400], lhsT=qt[:, half, :],
                    rhs=kp[:, half, :], start=True, stop=True)
            P[j]["stt"] = stt
        if lt < 3:
            l0, lsz = LT[lt]
            rs = r_p.tile([128, 8], FP32, tag="rs")
            for g in range(4):
                st = ps.tile([128, 2, 512], FP32, tag="ps")
                for i in range(2):
                    h = 2 * g + i
                    ab = 32 * (h % 4)
                    nc.tensor.matmul(
                        st[:lsz, i, 0:400],
                        lhsT=qp[ab:ab + 32, h // 4, l0:l0 + lsz],
                        rhs=kp[ab:ab + 32, h // 4, :],
                        start=True, stop=True, tile_position=(ab, 0))
                et = e_p.tile([128, 2, 400], BF16, tag="e")
                nc.scalar.activation(out=et[:lsz, :, :],
                                     in_=st[:lsz, :, 0:400], func=ACTF.Exp)
                for i in range(2):
                    h = 2 * g + i
                    P[j]["esbs"][(lt, h)] = et[:, i, :]
                    nc.vector.tensor_scalar(
                        out=scr[:lsz, :], in0=et[:lsz, i, :], scalar1=0.0,
                        scalar2=0.0, op0=ALU.add, op1=ALU.add,
                        accum_out=rs[:lsz, h:h + 1])
            ri = r_p.tile([128, 8], BF16, tag="ri")
            nc.vector.reciprocal(ri[:lsz, :], rs[:lsz, :])
            P[j]["rinvs"][lt] = ri
        else:
            # Tail block: one exp covers both halves' packed tails. PSUM rows
            # 64..128 of the pstail banks are never written (they stay 0 from
            # reset, exp gives 1.0) and their rinv rows are memset to zero so
            # the colsum ignores them.
            stt = P[j]["stt"]
            et = e_p.tile([128, 2, 400], BF16, tag="e")
            nc.scalar.activation(out=et[:], in_=stt[:, :, 0:400], func=ACTF.Exp)
            P[j]["etail"] = et
            rit = r_p.tile([128, 2], BF16, tag="rit")
            nc.vector.memset(rit[:], 0.0)
            rst = r_p.tile([128, 2], FP32, tag="rst")
            for half in range(2):
                nc.vector.tensor_scalar(
                    out=scr[0:64, :], in0=et[0:64, half, :], scalar1=0.0,
                    scalar2=0.0, op0=ALU.add, op1=ALU.add,
                    accum_out=rst[0:64, half:half + 1])
            nc.vector.reciprocal(rit[0:64, :], rst[0:64, :])
            P[j]["ritail"] = rit

    def emit_colsums(j, lt):
        row = pcbt[j // 4][32 * (j % 4):32 * (j % 4) + 1, 0:400]
        tp = (0, 32 * (j % 4))
        if lt < 3:
            l0, lsz = LT[lt]
            for h in range(NHEADS):
                nc.tensor.matmul(
                    row, lhsT=P[j]["rinvs"][lt][:lsz, h:h + 1],
                    rhs=P[j]["esbs"][(lt, h)][:lsz, :],
                    start=(lt == 0 and h == 0), stop=False, tile_position=tp)
        else:
            for half in range(2):
                nc.tensor.matmul(
                    row, lhsT=P[j]["ritail"][:, half:half + 1],
                    rhs=P[j]["etail"][:, half, :],
                    start=False, stop=(half == 1), tile_position=tp)
            del P[j]

    blocks = [(j, lt) for j in range(8) for lt in range(4)]
    n = len(blocks)
    emit_xq(0)
    emit_xq(1)
    nc.sync.dma_start(ukA[:, :, 1:4], ukeyd[:, :, 1:4])
    nc.sync.dma_start(ukB[:], ukeyd[:, :, 4:8])
    for part in range(4):
        emit_proj_part(0, part, use_ps=True)
    for idx in range(n + lag):
        if idx < n:
            j, lt = blocks[idx]
            # Patch j+1's projection part comes BEFORE the block so its DVE
            # move isn't queued behind the block's rowsums — otherwise patch
            # j+1's first scores (hence ACT) stall on the late kp at every
            # patch boundary. xq is prefetched two patches ahead so the proj
            # matmul never waits on the DMA.
            if j + 1 < 8:
                emit_proj_part(j + 1, lt)
            if lt == 2 and j + 2 < 8:
                emit_xq(j + 2)
            emit_block(j, lt)
        if lag <= idx:
            jc, ltc = blocks[idx - lag]
            emit_colsums(jc, ltc)
            if ltc == 3:
                t, q = jc // 4, jc % 4
                nc.vector.tensor_scalar_add(
                    stage[32 * q:32 * q + 1, t, :],
                    pcbt[t][32 * q:32 * q + 1, 0:400], 0.0)
                if jc == 7:
                    src = stage.rearrange("(q s) t f -> q s t f", s=32)[:, 0]
                    nc.sync.dma_start(outp, src)


def build(reps=1, lag=2):
    nc = bacc.Bacc("TRN2", target_bir_lowering=False, debug=False, num_devices=8)
    xqd = nc.dram_tensor("xq", (8, 128, 2, 400), FP8, kind="ExternalInput").ap()
    ukeyd = nc.dram_tensor("ukey", (128, 2, 8, 400), FP8, kind="ExternalInput").ap()
    wqkd = nc.dram_tensor("wqk", (128, 2, 512), FP8, kind="ExternalInput").ap()
    miscd = nc.dram_tensor("misc", (128, 260), R32, kind="ExternalInput").ap()
    outp = nc.dram_tensor("out", (4, 2, 400), FP32, kind="ExternalOutput").ap()
    aps = (xqd, ukeyd, wqkd, miscd, outp)

    with tile.TileContext(nc) as tc:
        with (
            tc.tile_pool(name="const", bufs=1) as const,
            tc.tile_pool(name="xq", bufs=3) as xq_p,
            tc.tile_pool(name="qk", bufs=4) as qk_p,
            tc.tile_pool(name="e", bufs=17) as e_p,
            tc.tile_pool(name="r", bufs=8) as r_p,
            tc.tile_pool(name="ps", bufs=2, space="PSUM") as ps,
            tc.tile_pool(name="pt", bufs=1, space="PSUM") as pt_p,
            tc.tile_pool(name="pcb", bufs=1, space="PSUM") as pcb,
        ):
            pools = (const, xq_p, qk_p, e_p, r_p, ps, pt_p, pcb)
            with nc.allow_low_precision(reason="fp8/bf16/fp32r attention"):
                if reps == 1:
                    _body(nc, tc, pools, aps, lag)
                else:
                    with tc.For_i(0, reps, 1):
                        _body(nc, tc, pools, aps, lag)
    nc.compile()
    return nc


# ----------------------------------------------------------------------------
# Entry point
# ----------------------------------------------------------------------------

def kernel(**inputs) -> np.ndarray:
    in_maps, post = prepare_inputs(
        inputs["x"], inputs["in_proj_w"], inputs["in_proj_b"],
        inputs["sal_w"], inputs["sal_b"])
    nc = build(reps=1)
    res = run_bass_kernel_spmd(nc, in_maps, core_ids=list(range(8)))
    return finish_output([r["out"] for r in res.results], post).astype(np.float32)


if __name__ == "__main__":
    rng = np.random.default_rng(0)
    ins = {
        "x": rng.standard_normal((5, 256, 80, 80), dtype=np.float32),
        "in_proj_w": (rng.standard_normal((768, 256)) * 0.05).astype(np.float32),
        "in_proj_b": (rng.standard_normal(768) * 0.05).astype(np.float32),
        "sal_w": rng.standard_normal(1).astype(np.float32),
        "sal_b": rng.standard_normal(1).astype(np.float32),
    }
    out = kernel(**ins)
    print("kernel out:", out.shape, out.dtype, float(np.abs(out).mean()))


# revision 51
# speedup vs baseline: 1.1848x; 1.0092x over previous
"""Trainium2 Bass kernel for nn_DWMF_51874615001833 (sparse patch attention).

Computation (matches the reference nn.Module):
  - x[0:4]  : 4 local images [256, 80, 80], split into 64 patches of [256, 20, 20]
  - x[4]    : global image, bilinear-upsampled 2x and split into an 8x8 patch grid
  - per patch: head-averaged, query-averaged attention weights between the
    (local patch + sine pos-embed) queries and (global patch + pos-embed) keys
    produce a [20, 20] importance map; the local patch is scaled by
    0.6 * sigmoid(sal_w * imp + sal_b), patches are re-assembled, plus
    0.5 * upsampled-global.

Device/host split: the device computes ONLY the attention reduction
  colsum[s] = sum_h sum_l exp(score_h[l, s]) / rowsum_h[l]
per patch (the compute-bound core: 64 x 8 heads x 400x400 softmax weights).
Everything that is cheap elementwise host work rides on the host: sine pos
embedding (folded into the inputs), the 2x bilinear upsample of the global
image (the key patches are shipped pre-upsampled), the final
sigmoid / multiply / merge. The device inputs are fp8e4 (validated: final
rel err ~3e-6 vs fp32 reference), which also halves/quarters DMA traffic.

Sharding: core d handles patches n = 8d..8d+7 (data parallel), which attend
to row d of the 8x8 global key grid. Projection weights are replicated.

Per-core device pipeline (8 patches):
  - qp/kp = W^T @ (x + pos) + b : one fp8 DoubleRow matmul per (proj, mt)
    half (contracts all 256 input dims in one instruction at 0.5 cyc/row),
    bias added during the PSUM->SBUF move (DVE tensor_scalar with
    per-partition scalar), output fp32r so the score matmuls self-load.
  - scores per (l-tile, head): PE fp32r matmuls, K=32, quadrant-packed via
    tile_position, two heads per [128, 2, 512] PSUM ring tile (3-deep ring,
    shared with the projection and tail tiles: exactly 6 of 8 PSUM banks).
  - the 16-row l-tail of all 8 heads is packed into ONE [128, 400] bank by
    two ACCUMULATING M=128 matmuls (per e-half, zero-padded lhsT columns
    from the DMA'd qt scaffold stack the halves at rows 64*half+16*hq+l),
    so the whole tail costs one exp / rowsum / recip / colsum-matvec.
  - exp on ACT (two heads per instruction), bf16 out - ACT is the bottleneck
    engine (~87 us busy of ~100 us total), so exp instruction count is
    minimized: 12 pair tiles + 1 packed tail per patch.
  - rowsums on DVE (tensor_scalar accum_out, 4x perf mode: all-SBUF bf16).
  - colsum via PE matvec accumulation (bf16), all (head, l-tile) terms into
    one [1, 400] PSUM row per patch; 4 patches share one PSUM bank at
    partitions {0, 32, 64, 96} (full-K (128, 32) tiles encode those bases).
  - one [4, 2, 400] DMA ships the raw colsums out; host applies the sigmoid
    and the final merge.
"""

import math

import numpy as np

import concourse.tile as tile
from concourse import bacc, mybir
from concourse.bass_utils import run_bass_kernel_spmd

FP32 = mybir.dt.float32
R32 = mybir.dt.float32r
BF16 = mybir.dt.bfloat16
FP8 = mybir.dt.float8e4
DR = mybir.MatmulPerfMode.DoubleRow


ALU = mybir.AluOpType
ACTF = mybir.ActivationFunctionType

D = 256
NHEADS = 8
PH = PW = 20
L = S = 400
LT = [(0, 128), (128, 128), (256, 128)]  # full l-tiles; 16-row tail packed


# ----------------------------------------------------------------------------
# Host-side preparation
# ----------------------------------------------------------------------------

def _pos_embed_sine(h, w, F):
    scale = 2.0 * math.pi
    eps = 1e-6
    y = (np.arange(1, h + 1, dtype=np.float64) - 0.5) / (h + eps) * scale
    x = (np.arange(1, w + 1, dtype=np.float64) - 0.5) / (w + eps) * scale
    i = np.arange(F, dtype=np.float64)
    dim_t = 10000.0 ** (2.0 * np.floor(i / 2.0) / F)
    px = x[:, None] / dim_t
    py = y[:, None] / dim_t

    def interleave(p):
        return np.stack(
            [np.sin(p[:, 0::2]), np.cos(p[:, 1::2])], axis=-1
        ).reshape(p.shape[0], -1)

    px = interleave(px)
    py = interleave(py)
    pos_y = np.broadcast_to(py[:, None, :], (h, w, F))
    pos_x = np.broadcast_to(px[None, :, :], (h, w, F))
    return np.concatenate([pos_y, pos_x], axis=-1).transpose(2, 0, 1)  # [2F,h,w]


def _up2(a):
    """Separable 2x bilinear upsample (align_corners=False): [C,H,W]->[C,2H,2W]."""
    C, H, W = a.shape
    pad = np.pad(a, ((0, 0), (1, 1), (1, 1)), mode="edge")
    h0 = 0.75 * pad[:, 1:1 + H, :] + 0.25 * pad[:, 0:H, :]
    h1 = 0.75 * pad[:, 1:1 + H, :] + 0.25 * pad[:, 2:2 + H, :]
    v = np.empty((C, 2 * H, W + 2), a.dtype)
    v[:, 0::2] = h0
    v[:, 1::2] = h1
    out = np.empty((C, 2 * H, 2 * W), a.dtype)
    out[:, :, 0::2] = 0.75 * v[:, :, 1:1 + W] + 0.25 * v[:, :, 0:W]
    out[:, :, 1::2] = 0.75 * v[:, :, 1:1 + W] + 0.25 * v[:, :, 2:2 + W]
    return out


def _fp8(a):
    return np.ascontiguousarray(a.astype(mybir.dt.np(FP8)))


def prepare_inputs(x, in_proj_w, in_proj_b, sal_w, sal_b):
    """Returns (in_maps list of 8 dicts, post ctx dict for finish_output)."""
    x = np.asarray(x, np.float32)
    in_proj_w = np.asarray(in_proj_w, np.float32)
    in_proj_b = np.asarray(in_proj_b, np.float32)
    inv = 1.0 / math.sqrt(D // NHEADS)

    pos = _pos_embed_sine(PH, PW, D // 2).reshape(D, L).astype(np.float32)
    Wq = in_proj_w[:D].astype(np.float64)
    Wk = in_proj_w[D:2 * D].astype(np.float64)
    bq = in_proj_b[:D].astype(np.float64)
    bk = in_proj_b[D:2 * D].astype(np.float64)

    # weights e-split [128(e-lo), 2(e-hi), 256(m)] for DoubleRow contraction
    wq_dev = _fp8((Wq.T * inv).reshape(2, 128, D).transpose(1, 0, 2))
    wk_dev = _fp8(Wk.T.reshape(2, 128, D).transpose(1, 0, 2))
    # per-feature bias columns [128(m-lo), 2(mt)]
    bqc = np.ascontiguousarray((bq * inv).reshape(2, 128).T.astype(np.float32))
    bkc = np.ascontiguousarray(bk.reshape(2, 128).T.astype(np.float32))

    loc = x[:4]
    upg = _up2(x[4])  # [256, 160, 160]
    locp = (loc.reshape(4, D, 4, PH, 4, PW).transpose(0, 2, 4, 1, 3, 5)
            .reshape(64, D, L))
    glbp = (upg.reshape(D, 8, PH, 8, PW).transpose(1, 3, 0, 2, 4)
            .reshape(64, D, L))

    # e-split + pos fold, per patch: [64, 128, 2, 400]
    qall = (locp + pos).reshape(64, 2, 128, L).transpose(0, 2, 1, 3)
    kall = (glbp + pos).reshape(64, 2, 128, L).transpose(0, 2, 1, 3)
    qall = _fp8(qall)
    kall = _fp8(kall)

    wqk = np.concatenate([wq_dev, wk_dev], axis=2)          # [128, 2, 512]
    misc = np.concatenate(
        [bqc, bkc, np.zeros((128, 256), np.float32)], axis=1)  # [128, 260]
    in_maps = []
    for d in range(8):
        boot = np.concatenate(
            [wqk, kall[8 * d], qall[8 * d]], axis=2)            # [128,2,1312]
        in_maps.append({
            "xq": np.ascontiguousarray(qall[8 * d:8 * d + 8]),  # [8,128,2,400]
            "ukey": np.ascontiguousarray(
                kall[8 * d:8 * d + 8].transpose(1, 2, 0, 3)),   # [128,2,8,400]
            "boot": np.ascontiguousarray(boot), "misc": misc,
        })
    post = {
        "locp": locp.reshape(64, D, PH, PW),
        "upg": upg,
        "sal_w": float(np.asarray(sal_w).reshape(-1)[0]),
        "sal_b": float(np.asarray(sal_b).reshape(-1)[0]),
    }
    return in_maps, post


def finish_output(per_core_outs, post):
    """per_core_outs: list of 8 arrays [4, 2, 400] (raw colsums, [q, t] =
    patch 4t+q) -> full output."""
    cs = np.empty((64, L), np.float64)
    for d in range(8):
        o = np.asarray(per_core_outs[d], np.float64)  # [4, 2, 400]
        cs[8 * d:8 * d + 8] = o.transpose(1, 0, 2).reshape(8, L)
    z = post["sal_w"] * cs / (NHEADS * L) + post["sal_b"]
    imp = (0.6 / (1.0 + np.exp(-z))).astype(np.float32)  # [64, 400]
    weighted = post["locp"] * imp.reshape(64, 1, PH, PW)
    loc_m = (weighted.reshape(4, 4, 4, D, PH, PW)
             .transpose(0, 3, 1, 4, 2, 5).reshape(4, D, 80, 80))
    merged = (loc_m.reshape(2, 2, 1, D, 80, 80)
              .transpose(2, 3, 0, 4, 1, 5).reshape(1, D, 160, 160))
    return merged + post["upg"][None] * 0.5


# ----------------------------------------------------------------------------
# Device kernel
# ----------------------------------------------------------------------------

def _body(nc, tc, pools, aps, lag=2):
    const, xq_p, qk_p, e_p, r_p, ps, pcb = pools
    xqd, ukeyd, bootd, miscd, outp = aps

    # Everything the first patch's projections need rides ONE boot DMA
    # (descriptor generation is serial, ~625ns per DMA): weights + key patch
    # 0 + query patch 0.
    boot = const.tile([128, 2, 1312], FP8, tag="boot")
    nc.sync.dma_start(boot[:], bootd)
    wq_s = boot[:, :, 0:256]
    wk_s = boot[:, :, 256:512]
    uk0 = boot[:, :, 512:912]
    xq0 = boot[:, :, 912:1312]
    ukA = const.tile([128, 2, 4, 400], FP8, tag="ukA")
    ukB = const.tile([128, 2, 4, 400], FP8, tag="ukB")
    # misc: [0:2] q bias cols, [2:4] k bias cols, [4:260] the zero-padded
    # tail-lhsT scaffold (fp32r memset is invalid ISA, so zeros ride the DMA)
    misc = const.tile([128, 260], R32, tag="misc")
    nc.scalar.dma_start(misc[:], miscd)
    bqc_s = misc[:, 0:2].bitcast(FP32)
    bkc_s = misc[:, 2:4].bitcast(FP32)
    qt = misc[:, 4:260].rearrange("p (a b) -> p a b", a=2)

    scr = const.tile([128, 400], BF16, tag="scr")  # write-only rowsum main out
    stage = const.tile([128, 2, 400], FP32, tag="stage")  # colsum staging

    pcbt = [pcb.tile([128, 512], FP32, tag="pcb", bufs=2, name=f"pcb{t}")
            for t in range(2)]

    P = {}  # per-patch state

    def emit_xq(j):
        if j == 0:
            xq = xq0.rearrange("p a (o f) -> p a o f", o=1)[:, :, 0]
        else:
            xq = xq_p.tile([128, 2, 400], FP8, tag="xq")
            nc.sync.dma_start(xq[:], xqd[j])
        P[j] = dict(xq=xq, esbs={}, rinvs={})

    def emit_proj_part(j, part, use_ps=False):
        # One (projection, mt) quarter; spread across the previous patch's
        # blocks. A single fp8 DoubleRow matmul contracts all 256 input dims.
        # Patch 0's parts draw PSUM from the still-empty score ring so they
        # pipeline in parallel banks instead of serializing on the single
        # pt bank (startup latency).
        is_q, mt = ((True, 0), (False, 0), (True, 1), (False, 1))[part]
        w_s, b_s = (wq_s, bqc_s) if is_q else (wk_s, bkc_s)
        if is_q:
            rhs = P[j]["xq"][:]
        elif j == 0:
            rhs = uk0
        else:
            uk = ukA if j < 4 else ukB
            rhs = uk[:, :, j % 4]
        pt_t = ps.tile([128, 2, 512], FP32, tag="ps", name="pt_t")[:, 0, :]
        nc.tensor.matmul(pt_t[:, 0:400], lhsT=w_s[:, :, 128 * mt:128 * mt + 128],
                         rhs=rhs, start=True, stop=True, perf_mode=DR)
        key = "qp" if is_q else "kp"
        if mt == 0:
            P[j][key] = qk_p.tile([128, 2, 400], R32, tag=key, name=key)
        # bias add during the PSUM->SBUF move (per-partition scalar)
        nc.vector.tensor_scalar(
            out=P[j][key][:, mt, :], in0=pt_t[:, 0:400],
            scalar1=b_s[:, mt:mt + 1], scalar2=None, op0=ALU.add)

    def emit_block(j, lt):
        qp, kp = P[j]["qp"], P[j]["kp"]
        if lt == 2:
            # Stage the zero-padded tail lhsT ahead of the tail block so the
            # copies clear the DVE queue early. Half h's heads map to output
            # rows 64*half + 16*hq + l; the other half's columns stay zero,
            # so the two tail matmuls can ACCUMULATE into one full-height
            # [128, 400] region at the safe (0, 0) tile position.
            for h in range(NHEADS):
                hq, half = h % 4, h // 4
                c0 = 64 * half + 16 * hq
                nc.vector.tensor_scalar_add(
                    qt[32 * hq:32 * hq + 32, half, c0:c0 + 16],
                    qp[32 * hq:32 * hq + 32, half, 384:400], 0.0)
        if lt < 3:
            l0, lsz = LT[lt]
            rs = r_p.tile([128, 8], FP32, tag="rs")
            for g in range(4):
                st = ps.tile([128, 2, 512], FP32, tag="ps")
                for i in range(2):
                    h = 2 * g + i
                    ab = 32 * (h % 4)
                    nc.tensor.matmul(
                        st[:lsz, i, 0:400],
                        lhsT=qp[ab:ab + 32, h // 4, l0:l0 + lsz],
                        rhs=kp[ab:ab + 32, h // 4, :],
                        start=True, stop=True, tile_position=(ab, 0))
                et = e_p.tile([128, 2, 400], BF16, tag="e")
                nc.scalar.activation(out=et[:lsz, :, :],
                                     in_=st[:lsz, :, 0:400], func=ACTF.Exp)
                for i in range(2):
                    h = 2 * g + i
                    P[j]["esbs"][(lt, h)] = et[:, i, :]
                    nc.vector.tensor_scalar(
                        out=scr[:lsz, :], in0=et[:lsz, i, :], scalar1=0.0,
                        scalar2=0.0, op0=ALU.add, op1=ALU.add,
                        accum_out=rs[:lsz, h:h + 1])
            ri = r_p.tile([128, 8], BF16, tag="ri")
            nc.vector.reciprocal(ri[:lsz, :], rs[:lsz, :])
            P[j]["rinvs"][lt] = ri
        else:
            # Tail block: two ACCUMULATING M=128 matmuls (one per e-half,
            # both at tile position (0,0)) build all 8 heads' tail scores in
            # one [128, 400] bank: row p = 64*half + 16*hq + (l-384). Each
            # half's lhsT is zero in the other half's columns, so the
            # accumulation stacks them. One exp / rowsum / recip covers all.
            stt = ps.tile([128, 2, 512], FP32, tag="ps", name="stt")
            for half in range(2):
                nc.tensor.matmul(
                    stt[:, 0, 0:400], lhsT=qt[:, half, :],
                    rhs=kp[:, half, :], start=(half == 0), stop=(half == 1))
            et = e_p.tile([128, 400], BF16, tag="etail")
            nc.scalar.activation(out=et[:], in_=stt[:, 0, 0:400],
                                 func=ACTF.Exp)
            P[j]["etail"] = et
            rit = r_p.tile([128, 1], BF16, tag="rit")
            rst = r_p.tile([128, 1], FP32, tag="rst")
            nc.vector.tensor_scalar(
                out=scr[:, 0:400], in0=et[:, :], scalar1=0.0,
                scalar2=0.0, op0=ALU.add, op1=ALU.add,
                accum_out=rst[:, 0:1])
            nc.vector.reciprocal(rit[:], rst[:])
            P[j]["ritail"] = rit

    def emit_colsums(j, lt):
        row = pcbt[j // 4][32 * (j % 4):32 * (j % 4) + 1, 0:400]
        tp = (0, 32 * (j % 4))
        if lt < 3:
            l0, lsz = LT[lt]
            for h in range(NHEADS):
                nc.tensor.matmul(
                    row, lhsT=P[j]["rinvs"][lt][:lsz, h:h + 1],
                    rhs=P[j]["esbs"][(lt, h)][:lsz, :],
                    start=(lt == 0 and h == 0), stop=False, tile_position=tp)
        else:
            nc.tensor.matmul(
                row, lhsT=P[j]["ritail"][:, 0:1], rhs=P[j]["etail"][:, :],
                start=False, stop=True, tile_position=tp)
            del P[j]

    blocks = [(j, lt) for j in range(8) for lt in range(4)]
    n = len(blocks)
    emit_xq(0)
    emit_xq(1)
    nc.sync.dma_start(ukA[:, :, 1:4], ukeyd[:, :, 1:4])
    nc.sync.dma_start(ukB[:], ukeyd[:, :, 4:8])
    for part in range(4):
        emit_proj_part(0, part, use_ps=True)
    for idx in range(n + lag):
        if idx < n:
            j, lt = blocks[idx]
            # Patch j+1's projection part comes BEFORE the block so its DVE
            # move isn't queued behind the block's rowsums — otherwise patch
            # j+1's first scores (hence ACT) stall on the late kp at every
            # patch boundary. xq is prefetched two patches ahead so the proj
            # matmul never waits on the DMA.
            if j + 1 < 8:
                emit_proj_part(j + 1, lt)
            if lt == 2 and j + 2 < 8:
                emit_xq(j + 2)
            emit_block(j, lt)
        if lag <= idx:
            jc, ltc = blocks[idx - lag]
            emit_colsums(jc, ltc)
            if ltc == 3:
                t, q = jc // 4, jc % 4
                nc.vector.tensor_scalar_add(
                    stage[32 * q:32 * q + 1, t, :],
                    pcbt[t][32 * q:32 * q + 1, 0:400], 0.0)
                if jc == 7:
                    src = stage.rearrange("(q s) t f -> q s t f", s=32)[:, 0]
                    nc.sync.dma_start(outp, src)


def build(reps=1, lag=2):
    nc = bacc.Bacc("TRN2", target_bir_lowering=False, debug=False, num_devices=8)
    xqd = nc.dram_tensor("xq", (8, 128, 2, 400), FP8, kind="ExternalInput").ap()
    ukeyd = nc.dram_tensor("ukey", (128, 2, 8, 400), FP8, kind="ExternalInput").ap()
    bootd = nc.dram_tensor("boot", (128, 2, 1312), FP8, kind="ExternalInput").ap()
    miscd = nc.dram_tensor("misc", (128, 260), R32, kind="ExternalInput").ap()
    outp = nc.dram_tensor("out", (4, 2, 400), FP32, kind="ExternalOutput").ap()
    aps = (xqd, ukeyd, bootd, miscd, outp)

    with tile.TileContext(nc) as tc:
        with (
            tc.tile_pool(name="const", bufs=1) as const,
            tc.tile_pool(name="xq", bufs=3) as xq_p,
            tc.tile_pool(name="qk", bufs=4) as qk_p,
            tc.tile_pool(name="e", bufs=17) as e_p,
            tc.tile_pool(name="r", bufs=8) as r_p,
            tc.tile_pool(name="ps", bufs=3, space="PSUM") as ps,
            tc.tile_pool(name="pcb", bufs=1, space="PSUM") as pcb,
        ):
            pools = (const, xq_p, qk_p, e_p, r_p, ps, pcb)
            with nc.allow_low_precision(reason="fp8/bf16/fp32r attention"):
                if reps == 1:
                    _body(nc, tc, pools, aps, lag)
                else:
                    with tc.For_i(0, reps, 1):
                        _body(nc, tc, pools, aps, lag)
    nc.compile()
    return nc


# ----------------------------------------------------------------------------
# Entry point
# ----------------------------------------------------------------------------

def kernel(**inputs) -> np.ndarray:
    in_maps, post = prepare_inputs(
        inputs["x"], inputs["in_proj_w"], inputs["in_proj_b"],
        inputs["sal_w"], inputs["sal_b"])
    nc = build(reps=1)
    res = run_bass_kernel_spmd(nc, in_maps, core_ids=list(range(8)))
    return finish_output([r["out"] for r in res.results], post).astype(np.float32)


if __name__ == "__main__":
    rng = np.random.default_rng(0)
    ins = {
        "x": rng.standard_normal((5, 256, 80, 80), dtype=np.float32),
        "in_proj_w": (rng.standard_normal((768, 256)) * 0.05).astype(np.float32),
        "in_proj_b": (rng.standard_normal(768) * 0.05).astype(np.float32),
        "sal_w": rng.standard_normal(1).astype(np.float32),
        "sal_b": rng.standard_normal(1).astype(np.float32),
    }
    out = kernel(**ins)
    print("kernel out:", out.shape, out.dtype, float(np.abs(out).mean()))
